# revision 14
# baseline (speedup 1.0000x reference)
"""AttSTWNBlock Trainium2 kernel (v4).

Reference computation (B=2, C_IN=32, C_OUT=64, N=4096, T=32, K=3):
    y = einsum('bfst,ksn->btknf', x, wavelets)
    z = einsum('btknf,kfo->btkno', y, upsamplings)
    a = einsum('btkno,ko->btkn', z, att_u)
    a = softmax((a - mean_k) / (std_k(ddof=1) + EPS), axis=k)
    out = einsum('btkn,btkno->bont', a, z)

Sharding: row-parallel over the wavelet output-node axis n — each of the 8
cores owns a 512-node slice of wavelets' last axis and produces the full
(B,T,C_OUT) for its nodes.  No cross-device communication needed.

The attention scores a and their softmax are tiny (K*BT*N fp32 = 3 MB) but
numerically delicate: the (a-mu)/(std+eps) normalization divides by the
std over only K=3 values, which can be ~1e-3, so any low-precision noise
in a is amplified ~1000x through the softmax.  They are therefore computed
on the HOST in float64 (a 3x[64x4096 @ 4096x4096] gemm, ~0.3 s) and the
resulting softmax weights wt[k, bt, n] are shipped to the device.

The y/z path only feeds a convex combination (no amplification), so it
runs in fp16 end to end (validated: relmax err ~5e-4 vs fp32 reference,
tolerance 2e-2).

Per-core layout: c = (b, t, f) flattened to 2048 columns, 16 c-tiles of 128.
  Resident: all K*SB wavelet tiles (96 KB/partition fp16), streamed in as
  24 batched DMAs of [128, 2048].
  Prefix (fills the tensor engine while wavelets stream in):
    wtil[ct,k][(t4,f), n] = sel[ct].T @ wt[k]  (row-select + f-broadcast),
    copied to fp16 SBUF tiles (48 KB/partition).
  Per c-tile ct (ct0/ct1 MM1 interleaved s-wise to track W arrivals):
    MM1: psum_y[k][c(128), n(512)] += xT[s, ct].T @ W_k[s, :]  (32 s-blocks)
    DVE: wy_k = wtil[ct,k] * y_k   -> fp16
    out: po[hh][(t2,o), n] += uu[k,hh].T @ wy_k ; fp16 copy; DMA out rows
"""

import numpy as np

B, C_IN, C_OUT, N, T, K = 2, 32, 64, 4096, 32, 3
EPS = 5e-5
P = 128
S = N                    # contraction (source-node) dim
NCORES = 8
NS = N // NCORES         # nodes per core = 512
C = B * T * C_IN         # 2048 fused (b,t,f) columns
CT = C // P              # 16 c-tiles
SB = S // P              # 32 s-blocks
BT = B * T               # 64

_CACHE = {}


def _build_program(reps: int = 1):
    from contextlib import ExitStack

    import concourse.tile as tile
    from concourse import bacc, mybir

    f32 = mybir.dt.float32
    f16 = mybir.dt.float16

    nc = bacc.Bacc("TRN2", target_bir_lowering=False, debug=False)

    xt_d = nc.dram_tensor("xt", [CT, SB // 8, P, 8 * P], f16, kind="ExternalInput").ap()
    wv_d = nc.dram_tensor("wv", [K, SB // 4, P, 4 * NS], f16, kind="ExternalInput").ap()
    # packed constants: one DMA each.  selwt = [sel(CT*P) | wt(K*NS)] on 64
    # partitions; uu = K*2 blocks of [128, 128]
    selwt_d = nc.dram_tensor(
        "selwt", [BT, CT * P + K * NS], f16, kind="ExternalInput"
    ).ap()
    uu_d = nc.dram_tensor("uu", [P, K * 2 * P], f16, kind="ExternalInput").ap()
    out_d = nc.dram_tensor("out", [BT * C_OUT, NS], f16, kind="ExternalOutput").ap()

    def mm(ps, lhsT, rhs, start, stop):
        nc.tensor.matmul(ps, lhsT, rhs, start=start, stop=stop)

    with tile.TileContext(nc) as tc, ExitStack() as ctx:
        const = ctx.enter_context(tc.tile_pool(name="const", bufs=1))
        wpool = ctx.enter_context(tc.tile_pool(name="w", bufs=1))
        wtpool = ctx.enter_context(tc.tile_pool(name="wtil", bufs=1))
        xpool = ctx.enter_context(tc.tile_pool(name="x", bufs=2))
        ypool = ctx.enter_context(tc.tile_pool(name="y", bufs=2))
        wypool = ctx.enter_context(tc.tile_pool(name="wy", bufs=2))
        opool = ctx.enter_context(tc.tile_pool(name="o", bufs=2))
        py = ctx.enter_context(tc.tile_pool(name="py", bufs=1, space="PSUM"))
        pout = ctx.enter_context(tc.tile_pool(name="pout", bufs=2, space="PSUM"))

        # constants first (gpsimd DMA queue): two batched DMAs, needed by
        # the prefix sel-MMs
        selwt = const.tile([BT, CT * P + K * NS], f16, tag="selwt", name="selwt")
        nc.gpsimd.dma_start(selwt[:], selwt_d)
        uub = const.tile([P, K * 2 * P], f16, tag="uub", name="uub")
        nc.gpsimd.dma_start(uub[:], uu_d)
        sel_sb = {ct: selwt[:, ct * P : (ct + 1) * P] for ct in range(CT)}
        wt_sb = {
            k: selwt[:, CT * P + k * NS : CT * P + (k + 1) * NS] for k in range(K)
        }
        uu_sb = {
            (k, hh): uub[:, (k * 2 + hh) * P : (k * 2 + hh + 1) * P]
            for k in range(K)
            for hh in range(2)
        }

        # resident wavelet slice: 24 batched DMAs of [128, 2048] fp16
        # (4 s-blocks each), g-major so ct0's s-ordered accumulation can
        # chase the stream
        wg_sb = {}
        for g in range(SB // 4):
            for k in range(K):
                t = wpool.tile([P, 4 * NS], f16, tag=f"w{k}_{g}", name=f"w{k}_{g}")
                nc.sync.dma_start(t[:], wv_d[k, g])
                wg_sb[k, g] = t
        w_sb = {
            (k, s): wg_sb[k, s // 4][:, (s % 4) * NS : (s % 4 + 1) * NS]
            for k in range(K)
            for s in range(SB)
        }

        # prefix: broadcast softmax weights for every (ct, k) into fp16
        # SBUF tiles; rides the tensor engine while wavelets stream in
        wtil = {}
        for ct in range(CT):
            for k in range(K):
                pb = pout.tile([P, NS], f32, tag="po", name="po")
                mm(pb[:], sel_sb[ct][:], wt_sb[k][:], True, True)
                t = wtpool.tile([P, NS], f16, tag=f"wtil{ct}_{k}", name=f"wtil{ct}_{k}")
                nc.scalar.copy(t[:], pb[:])
                wtil[ct, k] = t

        def emit_x(ct):
            xgs = []
            for g in range(SB // 8):
                t = xpool.tile([P, 8 * P], f16, tag=f"x{g}", name=f"x{g}")
                nc.scalar.dma_start(t[:], xt_d[ct, g])
                xgs.append(t)
            return [
                xgs[s // 8][:, (s % 8) * P : (s % 8 + 1) * P] for s in range(SB)
            ]

        def emit_mm1(ct, xts):
            pss = [
                py.tile([P, NS], f32, tag=f"py{k}_{ct % 2}", name=f"py{k}_{ct % 2}")
                for k in range(K)
            ]
            for s in range(SB):
                for k in range(K):
                    mm(pss[k][:], xts[s], w_sb[k, s], s == 0, s == SB - 1)
            return pss

        def emit_tail(ct, pss):
            ys = [None] * K
            for k in range(K):
                y_sb = ypool.tile([P, NS], f32, tag=f"y{k}", name=f"y{k}")
                if k == K - 1:
                    nc.vector.tensor_copy(y_sb[:], pss[k][:])
                else:
                    nc.scalar.copy(y_sb[:], pss[k][:])
                ys[k] = y_sb
            wys = []
            for k in range(K):
                wy = wypool.tile([P, NS], f16, tag=f"wy{k}", name=f"wy{k}")
                nc.vector.tensor_mul(wy[:], wtil[ct, k][:], ys[k][:])
                wys.append(wy)
            b, tg = ct // 8, ct % 8
            for hh in range(2):
                po = pout.tile([P, NS], f32, tag="po", name="po")
                for k in range(K):
                    mm(po[:], uu_sb[k, hh][:], wys[k][:], k == 0, k == K - 1)
                o_sb = opool.tile([P, NS], f16, tag="o", name="o")
                nc.scalar.copy(o_sb[:], po[:])
                r0 = (b * T + tg * 4 + hh * 2) * C_OUT
                nc.sync.dma_start(out_d[r0 : r0 + P, :], o_sb[:])

        for rep in range(reps):
            # ct0 + ct1 interleaved s-wise: MM1 work tracks the wavelet
            # stream so the tensor engine isn't starved during the load
            xts0 = emit_x(0)
            xts1 = emit_x(1)
            ps0 = [
                py.tile([P, NS], f32, tag=f"py{k}_0", name=f"py{k}_0")
                for k in range(K)
            ]
            ps1 = [
                py.tile([P, NS], f32, tag=f"py{k}_1", name=f"py{k}_1")
                for k in range(K)
            ]
            for s in range(SB):
                for k in range(K):
                    mm(ps0[k][:], xts0[s], w_sb[k, s], s == 0, s == SB - 1)
                    mm(ps1[k][:], xts1[s], w_sb[k, s], s == 0, s == SB - 1)
            emit_tail(0, ps0)
            emit_tail(1, ps1)
            for ct in range(2, CT):
                xts = emit_x(ct)
                pss = emit_mm1(ct, xts)
                emit_tail(ct, pss)

    nc.compile()
    return nc


def _get_program(reps: int = 1):
    key = ("prog", reps)
    if key not in _CACHE:
        _CACHE[key] = _build_program(reps)
    return _CACHE[key]


def _host_weights(x, wavelets, upsamplings, att_u):
    """Exact (f64) attention softmax weights wt[k, bt, n]."""
    ua = np.einsum(
        "kfo,ko->kf", upsamplings.astype(np.float64), att_u.astype(np.float64)
    )
    # xu[k, s, bt] = sum_f x[b,f,s,t] * ua[k,f]
    xu = np.einsum("bfst,kf->ksbt", x.astype(np.float64), ua).reshape(K, S, BT)
    a = np.empty((K, BT, N))
    for k in range(K):
        a[k] = xu[k].T @ wavelets[k].astype(np.float64)
    mu = a.mean(axis=0, keepdims=True)
    std = np.sqrt(((a - mu) ** 2).sum(axis=0, keepdims=True) / (K - 1))
    an = (a - mu) / (std + EPS)
    e = np.exp(an - an.max(axis=0, keepdims=True))
    return (e / e.sum(axis=0, keepdims=True)).astype(np.float32)  # K, BT, N


def _host_inputs(x, wavelets, upsamplings, att_u):
    # xT[s, c] with c = (b, t, f); grouped 8 s-blocks per DMA tile:
    # [ct, g, p, (si q)] with si in 8, q in 128
    xt = x.transpose(2, 0, 3, 1).reshape(S, C)
    xt = np.ascontiguousarray(
        xt.reshape(SB // 8, 8, P, CT, P).transpose(3, 0, 2, 1, 4).reshape(
            CT, SB // 8, P, 8 * P
        )
    ).astype(np.float16)

    uu = np.zeros((P, K * 2 * P), np.float16)
    for k in range(K):
        for hh in range(2):
            for t2 in range(2):
                t4 = hh * 2 + t2
                uu[
                    t4 * 32 : (t4 + 1) * 32,
                    (k * 2 + hh) * P + t2 * 64 : (k * 2 + hh) * P + (t2 + 1) * 64,
                ] = upsamplings[k].astype(np.float16)
    sel = np.zeros((BT, CT * P), np.float16)
    for ct in range(CT):
        for t4 in range(4):
            sel[ct * 4 + t4, ct * P + t4 * 32 : ct * P + (t4 + 1) * 32] = 1.0

    wt = _host_weights(x, wavelets, upsamplings, att_u).astype(np.float16)

    in_maps = []
    for i in range(NCORES):
        # [K, SB//4, P, 4*NS]: 4 s-blocks batched per DMA tile
        wv = np.ascontiguousarray(
            wavelets[:, :, i * NS : (i + 1) * NS]
            .reshape(K, SB // 4, 4, P, NS)
            .transpose(0, 1, 3, 2, 4)
            .reshape(K, SB // 4, P, 4 * NS)
        ).astype(np.float16)
        # selwt = [sel(CT*P) | wt(K*NS)] on BT partitions
        wts = wt[:, :, i * NS : (i + 1) * NS]  # K, BT, NS
        selwt = np.concatenate(
            [sel] + [wts[k] for k in range(K)], axis=1
        )
        in_maps.append({"xt": xt, "wv": wv, "uu": uu, "selwt": selwt})
    return in_maps


def kernel(x, wavelets, upsamplings, att_u):
    from concourse.bass_utils import run_bass_kernel_spmd

    nc = _get_program()
    in_maps = _host_inputs(
        np.asarray(x, np.float32),
        np.asarray(wavelets, np.float32),
        np.asarray(upsamplings, np.float32),
        np.asarray(att_u, np.float32),
    )
    res = run_bass_kernel_spmd(nc, in_maps, list(range(NCORES)))
    full = np.concatenate(
        [res.results[i]["out"].astype(np.float32) for i in range(NCORES)], axis=1
    )
    return np.ascontiguousarray(
        full.reshape(B, T, C_OUT, N).transpose(0, 2, 3, 1)
    )


# revision 20
# speedup vs baseline: 1.0110x; 1.0110x over previous
"""AttSTWNBlock Trainium2 kernel (v4).

Reference computation (B=2, C_IN=32, C_OUT=64, N=4096, T=32, K=3):
    y = einsum('bfst,ksn->btknf', x, wavelets)
    z = einsum('btknf,kfo->btkno', y, upsamplings)
    a = einsum('btkno,ko->btkn', z, att_u)
    a = softmax((a - mean_k) / (std_k(ddof=1) + EPS), axis=k)
    out = einsum('btkn,btkno->bont', a, z)

Sharding: row-parallel over the wavelet output-node axis n — each of the 8
cores owns a 512-node slice of wavelets' last axis and produces the full
(B,T,C_OUT) for its nodes.  No cross-device communication needed.

The attention scores a and their softmax are tiny (K*BT*N fp32 = 3 MB) but
numerically delicate: the (a-mu)/(std+eps) normalization divides by the
std over only K=3 values, which can be ~1e-3, so any low-precision noise
in a is amplified ~1000x through the softmax.  They are therefore computed
on the HOST in float64 (a 3x[64x4096 @ 4096x4096] gemm, ~0.3 s) and the
resulting softmax weights wt[k, bt, n] are shipped to the device.

The y/z path only feeds a convex combination (no amplification), so it
runs in fp16 end to end (validated: relmax err ~5e-4 vs fp32 reference,
tolerance 2e-2).

Per-core layout: c = (b, t, f) flattened to 2048 columns, 16 c-tiles of 128.
  Resident: all K*SB wavelet tiles (96 KB/partition fp16), streamed in as
  24 batched DMAs of [128, 2048].
  Prefix (fills the tensor engine while wavelets stream in):
    wtil[ct,k][(t4,f), n] = sel[ct].T @ wt[k]  (row-select + f-broadcast),
    copied to fp16 SBUF tiles (48 KB/partition).
  Per c-tile ct (ct0/ct1 MM1 interleaved s-wise to track W arrivals):
    MM1: psum_y[k][c(128), n(512)] += xT[s, ct].T @ W_k[s, :]  (32 s-blocks)
    DVE: wy_k = wtil[ct,k] * y_k   -> fp16
    out: po[hh][(t2,o), n] += uu[k,hh].T @ wy_k ; fp16 copy; DMA out rows
"""

import numpy as np

B, C_IN, C_OUT, N, T, K = 2, 32, 64, 4096, 32, 3
EPS = 5e-5
P = 128
S = N                    # contraction (source-node) dim
NCORES = 8
NS = N // NCORES         # nodes per core = 512
C = B * T * C_IN         # 2048 fused (b,t,f) columns
CT = C // P              # 16 c-tiles
SB = S // P              # 32 s-blocks
BT = B * T               # 64

_CACHE = {}


def _build_program(reps: int = 1):
    from contextlib import ExitStack

    import concourse.tile as tile
    from concourse import bacc, mybir

    f32 = mybir.dt.float32
    f16 = mybir.dt.float16

    nc = bacc.Bacc("TRN2", target_bir_lowering=False, debug=False)

    xt_d = nc.dram_tensor("xt", [CT, SB // 8, P, 8 * P], f16, kind="ExternalInput").ap()
    wv_d = nc.dram_tensor("wv", [K, SB // 4, P, 4 * NS], f16, kind="ExternalInput").ap()
    # packed constants: one DMA each; wt first (prefix-critical).
    # uu = K*2 blocks of [128, 128]
    wtc_d = nc.dram_tensor("wtc", [BT, K * NS], f16, kind="ExternalInput").ap()
    selc_d = nc.dram_tensor("selc", [BT, CT * P], f16, kind="ExternalInput").ap()
    uu_d = nc.dram_tensor("uu", [P, K * 2 * P], f16, kind="ExternalInput").ap()
    out_d = nc.dram_tensor("out", [BT * C_OUT, NS], f16, kind="ExternalOutput").ap()

    def mm(ps, lhsT, rhs, start, stop):
        nc.tensor.matmul(ps, lhsT, rhs, start=start, stop=stop)

    with tile.TileContext(nc) as tc, ExitStack() as ctx:
        const = ctx.enter_context(tc.tile_pool(name="const", bufs=1))
        wpool = ctx.enter_context(tc.tile_pool(name="w", bufs=1))
        wtpool = ctx.enter_context(tc.tile_pool(name="wtil", bufs=1))
        xpool = ctx.enter_context(tc.tile_pool(name="x", bufs=2))
        ypool = ctx.enter_context(tc.tile_pool(name="y", bufs=2))
        wypool = ctx.enter_context(tc.tile_pool(name="wy", bufs=2))
        opool = ctx.enter_context(tc.tile_pool(name="o", bufs=2))
        py = ctx.enter_context(tc.tile_pool(name="py", bufs=1, space="PSUM"))
        pout = ctx.enter_context(tc.tile_pool(name="pout", bufs=2, space="PSUM"))

        # constants first (gpsimd DMA queue): three batched DMAs, wt first
        # (prefix-critical)
        wtc = const.tile([BT, K * NS], f16, tag="wtc", name="wtc")
        nc.gpsimd.dma_start(wtc[:], wtc_d)
        selc = const.tile([BT, CT * P], f16, tag="selc", name="selc")
        nc.gpsimd.dma_start(selc[:], selc_d)
        uub = const.tile([P, K * 2 * P], f16, tag="uub", name="uub")
        nc.gpsimd.dma_start(uub[:], uu_d)
        sel_sb = {ct: selc[:, ct * P : (ct + 1) * P] for ct in range(CT)}
        wt_sb = {k: wtc[:, k * NS : (k + 1) * NS] for k in range(K)}
        uu_sb = {
            (k, hh): uub[:, (k * 2 + hh) * P : (k * 2 + hh + 1) * P]
            for k in range(K)
            for hh in range(2)
        }

        # resident wavelet slice: 24 batched DMAs of [128, 2048] fp16
        # (4 s-blocks each), g-major so ct0's s-ordered accumulation can
        # chase the stream
        wg_sb = {}
        for g in range(SB // 4):
            for k in range(K):
                t = wpool.tile([P, 4 * NS], f16, tag=f"w{k}_{g}", name=f"w{k}_{g}")
                nc.sync.dma_start(t[:], wv_d[k, g])
                wg_sb[k, g] = t
        w_sb = {
            (k, s): wg_sb[k, s // 4][:, (s % 4) * NS : (s % 4 + 1) * NS]
            for k in range(K)
            for s in range(SB)
        }

        # prefix: broadcast softmax weights for every (ct, k) into fp16
        # SBUF tiles; rides the tensor engine while wavelets stream in
        wtil = {}
        for ct in range(CT):
            for k in range(K):
                j = (ct * K + k) % 8
                # rotate across all 8 psum banks (the py banks are free
                # until ct0/ct1's MM1) and alternate scalar/vector copies,
                # so the prefix drains at matmul rate, not ACT-copy rate
                if j < 6:
                    pb = py.tile(
                        [P, NS], f32, tag=f"py{j % 3}_{j // 3}", name="pbj"
                    )
                else:
                    pb = pout.tile([P, NS], f32, tag="po", name="pbj")
                mm(pb[:], sel_sb[ct], wt_sb[k], True, True)
                t = wtpool.tile([P, NS], f16, tag=f"wtil{ct}_{k}", name=f"wtil{ct}_{k}")
                if j % 2 == 0:
                    nc.scalar.copy(t[:], pb[:])
                else:
                    nc.vector.tensor_copy(t[:], pb[:])
                wtil[ct, k] = t

        def emit_x(ct):
            xgs = []
            for g in range(SB // 8):
                t = xpool.tile([P, 8 * P], f16, tag=f"x{g}", name=f"x{g}")
                nc.scalar.dma_start(t[:], xt_d[ct, g])
                xgs.append(t)
            return [
                xgs[s // 8][:, (s % 8) * P : (s % 8 + 1) * P] for s in range(SB)
            ]

        def emit_mm1(ct, xts):
            pss = [
                py.tile([P, NS], f32, tag=f"py{k}_{ct % 2}", name=f"py{k}_{ct % 2}")
                for k in range(K)
            ]
            for s in range(SB):
                for k in range(K):
                    mm(pss[k][:], xts[s], w_sb[k, s], s == 0, s == SB - 1)
            return pss

        def emit_tail(ct, pss):
            ys = [None] * K
            for k in range(K):
                y_sb = ypool.tile([P, NS], f32, tag=f"y{k}", name=f"y{k}")
                if k == K - 1:
                    nc.vector.tensor_copy(y_sb[:], pss[k][:])
                else:
                    nc.scalar.copy(y_sb[:], pss[k][:])
                ys[k] = y_sb
            wys = []
            for k in range(K):
                wy = wypool.tile([P, NS], f16, tag=f"wy{k}", name=f"wy{k}")
                nc.vector.tensor_mul(wy[:], wtil[ct, k][:], ys[k][:])
                wys.append(wy)
            b, tg = ct // 8, ct % 8
            for hh in range(2):
                po = pout.tile([P, NS], f32, tag="po", name="po")
                for k in range(K):
                    mm(po[:], uu_sb[k, hh], wys[k][:], k == 0, k == K - 1)
                o_sb = opool.tile([P, NS], f16, tag="o", name="o")
                nc.scalar.copy(o_sb[:], po[:])
                r0 = (b * T + tg * 4 + hh * 2) * C_OUT
                nc.sync.dma_start(out_d[r0 : r0 + P, :], o_sb[:])

        for rep in range(reps):
            # ct0 + ct1 interleaved s-wise: MM1 work tracks the wavelet
            # stream so the tensor engine isn't starved during the load
            xts0 = emit_x(0)
            xts1 = emit_x(1)
            ps0 = [
                py.tile([P, NS], f32, tag=f"py{k}_0", name=f"py{k}_0")
                for k in range(K)
            ]
            ps1 = [
                py.tile([P, NS], f32, tag=f"py{k}_1", name=f"py{k}_1")
                for k in range(K)
            ]
            for s in range(SB):
                for k in range(K):
                    mm(ps0[k][:], xts0[s], w_sb[k, s], s == 0, s == SB - 1)
                    mm(ps1[k][:], xts1[s], w_sb[k, s], s == 0, s == SB - 1)
            emit_tail(0, ps0)
            emit_tail(1, ps1)
            for ct in range(2, CT):
                xts = emit_x(ct)
                pss = emit_mm1(ct, xts)
                emit_tail(ct, pss)

    nc.compile()
    return nc


def _get_program(reps: int = 1):
    key = ("prog", reps)
    if key not in _CACHE:
        _CACHE[key] = _build_program(reps)
    return _CACHE[key]


def _host_weights(x, wavelets, upsamplings, att_u):
    """Exact (f64) attention softmax weights wt[k, bt, n]."""
    ua = np.einsum(
        "kfo,ko->kf", upsamplings.astype(np.float64), att_u.astype(np.float64)
    )
    # xu[k, s, bt] = sum_f x[b,f,s,t] * ua[k,f]
    xu = np.einsum("bfst,kf->ksbt", x.astype(np.float64), ua).reshape(K, S, BT)
    a = np.empty((K, BT, N))
    for k in range(K):
        a[k] = xu[k].T @ wavelets[k].astype(np.float64)
    mu = a.mean(axis=0, keepdims=True)
    std = np.sqrt(((a - mu) ** 2).sum(axis=0, keepdims=True) / (K - 1))
    an = (a - mu) / (std + EPS)
    e = np.exp(an - an.max(axis=0, keepdims=True))
    return (e / e.sum(axis=0, keepdims=True)).astype(np.float32)  # K, BT, N


def _host_inputs(x, wavelets, upsamplings, att_u):
    # xT[s, c] with c = (b, t, f); grouped 8 s-blocks per DMA tile:
    # [ct, g, p, (si q)] with si in 8, q in 128
    xt = x.transpose(2, 0, 3, 1).reshape(S, C)
    xt = np.ascontiguousarray(
        xt.reshape(SB // 8, 8, P, CT, P).transpose(3, 0, 2, 1, 4).reshape(
            CT, SB // 8, P, 8 * P
        )
    ).astype(np.float16)

    uu = np.zeros((P, K * 2 * P), np.float16)
    for k in range(K):
        for hh in range(2):
            for t2 in range(2):
                t4 = hh * 2 + t2
                uu[
                    t4 * 32 : (t4 + 1) * 32,
                    (k * 2 + hh) * P + t2 * 64 : (k * 2 + hh) * P + (t2 + 1) * 64,
                ] = upsamplings[k].astype(np.float16)
    sel = np.zeros((BT, CT * P), np.float16)
    for ct in range(CT):
        for t4 in range(4):
            sel[ct * 4 + t4, ct * P + t4 * 32 : ct * P + (t4 + 1) * 32] = 1.0

    wt = _host_weights(x, wavelets, upsamplings, att_u).astype(np.float16)

    in_maps = []
    for i in range(NCORES):
        # [K, SB//4, P, 4*NS]: 4 s-blocks batched per DMA tile
        wv = np.ascontiguousarray(
            wavelets[:, :, i * NS : (i + 1) * NS]
            .reshape(K, SB // 4, 4, P, NS)
            .transpose(0, 1, 3, 2, 4)
            .reshape(K, SB // 4, P, 4 * NS)
        ).astype(np.float16)
        wts = wt[:, :, i * NS : (i + 1) * NS]  # K, BT, NS
        wtc = np.ascontiguousarray(wts.transpose(1, 0, 2).reshape(BT, K * NS))
        in_maps.append({"xt": xt, "wv": wv, "uu": uu, "wtc": wtc, "selc": sel})
    return in_maps


def kernel(x, wavelets, upsamplings, att_u):
    from concourse.bass_utils import run_bass_kernel_spmd

    nc = _get_program()
    in_maps = _host_inputs(
        np.asarray(x, np.float32),
        np.asarray(wavelets, np.float32),
        np.asarray(upsamplings, np.float32),
        np.asarray(att_u, np.float32),
    )
    res = run_bass_kernel_spmd(nc, in_maps, list(range(NCORES)))
    full = np.concatenate(
        [res.results[i]["out"].astype(np.float32) for i in range(NCORES)], axis=1
    )
    return np.ascontiguousarray(
        full.reshape(B, T, C_OUT, N).transpose(0, 2, 3, 1)
    )
